# revision 1
# baseline (speedup 1.0000x reference)
"""Multi-head causal attention (B=2, T=2048, D=1024, H=16, Dh=64) on 8 TRN2
NeuronCores via Bass/Tile.

Sharding: core c -> (batch b = c//4, head group hg = c%4, heads 4*hg..4*hg+3).
Each core computes its 4 heads' attention for its batch plus the partial
output projection over those heads' dims; the host sums the 4 partials per
batch and adds the output bias.

Device-side layout (per core):
  - x^T [1024, 2048] staged in 8 SBUF chunks [128, 2048] (float32r)
  - Q^T, K^T computed directly transposed as 2 "pair" tiles [128=2*64, T]
  - V computed in [T, dims] orientation via an augmented weight matrix
    [1024, 4*65] whose 65th column per head is zero-weight/one-bias, so the
    packed V' tile (with softmax-denominator ones columns) comes straight
    out of the matmul
  - scores computed transposed S^T[k, q] = K @ Q^T so softmax normalization
    is the only partition-dim reduction (handled by the ones columns)
  - causal mask applied additively (-1e30) on PSUM before the exp
  - all matmuls run as float32r (full PE rate at free dim >= 256)
"""

import numpy as np

D_MODEL = 1024
N_HEADS = 16
D_HEAD = 64
B = 2
T = 2048
N_CORES = 8
HPC = 4          # heads per core
MPC = HPC * D_HEAD  # head dims per core = 256
MPCA = HPC * 65     # augmented V dims = 260
NQ = 4           # q blocks of 512
QB = 512
KB = 128         # k block
NTB = T // 128   # 16 T blocks for V / O
SCALE = 1.0 / 8.0  # 1/sqrt(D_HEAD)
NEG = -1.0e30

PROFILE = False
LAST_RESULTS = None

_CACHE = {}


def _split_waits(nc, mybir, max_waits=1):
    """This walrus build rejects instructions carrying more than `max_waits`
    semaphore waits. Move the excess onto InstNoOp carriers inserted just
    before the instruction on the same engine (same blocking semantics)."""
    for func in nc.m.functions:
        for bb in func.blocks:
            todo = [
                inst for inst in bb.instructions
                if inst.sync_info is not None
                and inst.sync_info.on_wait
                and len(inst.sync_info.on_wait) > max_waits
            ]
            if not todo:
                continue
            carriers = {}
            for inst in todo:
                si = inst.sync_info
                waits = list(si.on_wait)
                si.on_wait = waits[-max_waits:]
                excess = waits[:-max_waits]
                chunks = []
                for i in range(0, len(excess), max_waits):
                    chunk = excess[i: i + max_waits]
                    bi = nc.engines[inst.engine].nop(nofuse=True)
                    nop_inst = bi.ins
                    cur = nc.cur_bb.bb
                    assert cur.instructions[-1] is nop_inst
                    cur.instructions = cur.instructions[:-1]
                    nop_inst.sync_info = mybir.SyncInfo(on_wait=chunk, on_update=[])
                    chunks.append(nop_inst)
                carriers[id(inst)] = chunks
            new_list = []
            for inst in bb.instructions:
                new_list.extend(carriers.get(id(inst), ()))
                new_list.append(inst)
            bb.instructions = new_list


def _build_nc(with_bias=False):
    import concourse.bass as bass
    import concourse.mybir as mybir

    f32 = mybir.dt.float32
    f32r = mybir.dt.float32r
    Exp = mybir.ActivationFunctionType.Exp
    Identity = mybir.ActivationFunctionType.Identity

    nc = bass.Bass("TRN2", target_bir_lowering=False, debug=False, num_devices=N_CORES)

    xt_d = nc.dram_tensor("xt", [D_MODEL, T], f32r, kind="ExternalInput").ap()
    wq_d = nc.dram_tensor("wq", [D_MODEL, MPC], f32r, kind="ExternalInput").ap()
    wk_d = nc.dram_tensor("wk", [D_MODEL, MPC], f32r, kind="ExternalInput").ap()
    wv_d = nc.dram_tensor("wv", [D_MODEL, MPCA], f32r, kind="ExternalInput").ap()
    wo_d = nc.dram_tensor("wo", [MPC, D_MODEL], f32r, kind="ExternalInput").ap()
    bq_d = nc.dram_tensor("bqr", [1, MPC], f32r, kind="ExternalInput").ap()
    bk_d = nc.dram_tensor("bkr", [1, MPC], f32r, kind="ExternalInput").ap()
    bv_d = nc.dram_tensor("bv1", [1, MPCA], f32r, kind="ExternalInput").ap()
    ones_d = nc.dram_tensor("ones", [1, QB], f32r, kind="ExternalInput").ap()
    mk_d = nc.dram_tensor("masks", [4, 128, QB], f32, kind="ExternalInput").ap()
    out_d = nc.dram_tensor("out", [T, D_MODEL], f32, kind="ExternalOutput").ap()

    import concourse.tile as tile
    with tile.TileContext(nc) as tc:
        with tc.tile_pool(name="big", bufs=8) as big, \
             tc.tile_pool(name="med", bufs=1) as med, \
             tc.tile_pool(name="small", bufs=1) as small, \
             tc.tile_pool(name="ps", bufs=2, space="PSUM") as ps:

            # ---- static SBUF tensors ----
            wq_sb = med.tile([128, 8 * MPC], f32r, tag="w", bufs=4, name="wq_sb")
            wk_sb = med.tile([128, 8 * MPC], f32r, tag="w", bufs=4, name="wk_sb")
            wv_sb = med.tile([128, 8 * MPCA], f32r, tag="w", bufs=4, name="wv_sb")
            wo_sb = med.tile([128, 2 * D_MODEL], f32r, tag="w", bufs=4, name="wo_sb")
            mk_sb = med.tile([128, 4 * QB], f32, tag="mk", name="mk_sb")
            bq_sb = small.tile([1, MPC], f32r, tag="bq", name="bq_sb")
            bk_sb = small.tile([1, MPC], f32r, tag="bk", name="bk_sb")
            bv_sb = small.tile([1, MPCA], f32r, tag="bv", name="bv_sb")
            ones_sb = small.tile([1, QB], f32r, tag="ones", name="ones_sb")
            vp_sb = med.tile([128, NTB * MPCA], f32r, tag="vp", name="vp_sb")

            nc.sync.dma_start(
                wq_sb[:].rearrange("p (c m) -> p c m", c=8),
                wq_d.rearrange("(c p) m -> p c m", p=128),
            )
            nc.sync.dma_start(bq_sb[:], bq_d)
            nc.sync.dma_start(bk_sb[:], bk_d)
            nc.sync.dma_start(bv_sb[:], bv_d)
            nc.sync.dma_start(ones_sb[:], ones_d)

            # ---- per-quarter pipeline ----
            xt_view = xt_d.rearrange("(c p) t -> c p t", p=128)

            qt = [big.tile([128, T], f32r, tag="qt", bufs=4, name=f"qt{p}") for p in range(2)]
            kt = [big.tile([128, T], f32r, tag="qt", bufs=4, name=f"kt{p}") for p in range(2)]
            ctxu = [big.tile([128, T], f32r, tag="ctxu", bufs=2, name=f"ctxu{p}")
                    for p in range(2)]

            # software pipeline over t-quarters: x-DMA -> QK proj -> V' ->
            # attention q-block -> O-proj + store. Tile's dep tracking lets
            # quarter n+1's projections overlap quarter n's attention. x^T
            # tiles are quarter-local so their slots recycle.
            def emit_x(nt):
                nsl = slice(nt * QB, (nt + 1) * QB)
                xts = []
                for dc in range(8):
                    xt_t = big.tile([128, QB], f32r, tag="xtq", bufs=16,
                                    name=f"xt{nt}_{dc}")
                    nc.sync.dma_start(xt_t[:], xt_view[dc, :, nsl])
                    xts.append(xt_t)
                return xts

            def emit_proj(nt, xts):
                nsl = slice(nt * QB, (nt + 1) * QB)

                # Q^T / K^T projections for this quarter
                for (w_sb, dst, b_sb) in ((wq_sb, qt, bq_sb), (wk_sb, kt, bk_sb)):
                    for pt in range(2):
                        pp = ps.tile([128, QB], f32, tag="s2", bufs=3, name="qk_ps")
                        for dc in range(8):
                            nc.tensor.matmul(
                                pp[:],
                                w_sb[:, dc * MPC + pt * 128: dc * MPC + (pt + 1) * 128],
                                xts[dc][:],
                                start=(dc == 0),
                                stop=(dc == 7 and not with_bias),
                            )
                        if with_bias:
                            nc.tensor.matmul(
                                pp[:],
                                b_sb[0:1, pt * 128: (pt + 1) * 128],
                                ones_sb[0:1, :],
                                start=False, stop=True,
                            )
                        nc.scalar.copy(dst[pt][:, nsl], pp[:])

                # V' for this quarter's 4 T-blocks
                for tb in range(4 * nt, 4 * nt + 4):
                    ltsl = slice((tb - 4 * nt) * 128, (tb - 4 * nt + 1) * 128)
                    vps = ps.tile([128, MPCA], f32, tag="s2", bufs=3, name="v_ps")
                    for dc in range(8):
                        nc.tensor.matmul(
                            vps[:],
                            xts[dc][:, ltsl],
                            wv_sb[:, dc * MPCA: (dc + 1) * MPCA],
                            start=(dc == 0),
                            stop=False,
                        )
                    nc.tensor.matmul(
                        vps[:], ones_sb[0:1, 0:128], bv_sb[0:1, :],
                        start=False, stop=True,
                    )
                    nc.vector.tensor_copy(
                        vp_sb[:, tb * MPCA: (tb + 1) * MPCA], vps[:])

            def emit_attn(qb):
                qsl = slice(qb * QB, (qb + 1) * QB)
                for hc in range(HPC):
                    pt, hl = hc // 2, hc % 2
                    hrow = slice(hl * 64, hl * 64 + 64)
                    ctx_ps = ps.tile([65, QB], f32, tag="ctx", bufs=2, name="ctx_ps")
                    # off-diagonal k-block pairs: full q width, no mask
                    for pj in range(2 * qb):
                        kb0 = 2 * pj
                        s_ps = ps.tile([128, 2 * QB], f32, tag="s2", bufs=3,
                                       name="s_ps")
                        for ki in range(2):
                            kb = kb0 + ki
                            nc.tensor.matmul(
                                s_ps[:, ki * QB: (ki + 1) * QB],
                                kt[pt][hrow, kb * KB: (kb + 1) * KB],
                                qt[pt][hrow, qsl],
                                start=True, stop=True,
                            )
                        p_sb = big.tile([128, 2 * QB], f32r, tag="p", bufs=4,
                                        name="p_sb")
                        nc.scalar.activation(p_sb[:], s_ps[:], Exp, scale=SCALE)
                        for ki in range(2):
                            kb = kb0 + ki
                            nc.tensor.matmul(
                                ctx_ps[:],
                                vp_sb[:, kb * MPCA + hc * 65: kb * MPCA + (hc + 1) * 65],
                                p_sb[:, ki * QB: (ki + 1) * QB],
                                start=(kb == 0), stop=False,
                            )
                    # diagonal band: narrowed to the unmasked q range.
                    # rels (0,1) pack as [0:512]+[512:896]; rels (2,3) as
                    # [0:256]+[256:384] - each matmul out stays in one PSUM
                    # bank and each pair shares a single exp.
                    for g in range(2):
                        rel_a = 2 * g
                        wa = QB - rel_a * KB
                        wb = QB - (rel_a + 1) * KB
                        off_b = wa  # 512 for g=0 (bank 1), 256 for g=1 (bank 0)
                        s_d = ps.tile([128, 2 * QB], f32, tag="s2", bufs=3,
                                      name="s_d")
                        for ri, (rel, w, off) in enumerate(
                                ((rel_a, wa, 0), (rel_a + 1, wb, off_b))):
                            kb = 4 * qb + rel
                            qoff = qb * QB + rel * KB
                            nc.tensor.matmul(
                                s_d[:, off: off + w],
                                kt[pt][hrow, kb * KB: (kb + 1) * KB],
                                qt[pt][hrow, qoff: qoff + w],
                                start=True, stop=True,
                            )
                            nc.vector.tensor_add(
                                s_d[:, off: off + KB], s_d[:, off: off + KB],
                                mk_sb[:, rel * QB + rel * KB: rel * QB + rel * KB + KB],
                            )
                        p_d = big.tile([128, 2 * QB], f32r, tag="p", bufs=4,
                                       name="p_d")
                        nc.scalar.activation(p_d[:, 0: off_b + wb],
                                             s_d[:, 0: off_b + wb], Exp,
                                             scale=SCALE)
                        for rel, w, off in ((rel_a, wa, 0), (rel_a + 1, wb, off_b)):
                            kb = 4 * qb + rel
                            nc.tensor.matmul(
                                ctx_ps[:, rel * KB: QB],
                                vp_sb[:, kb * MPCA + hc * 65: kb * MPCA + (hc + 1) * 65],
                                p_d[:, off: off + w],
                                start=(kb == 0), stop=(kb == 4 * qb + 3),
                            )
                    # stash unnormalized ctx + sums; normalize in place
                    nc.vector.tensor_copy(ctxu[pt][hrow, qsl], ctx_ps[0:64, :])
                    srow = big.tile([1, QB], f32r, tag="srow", bufs=4, name="srow")
                    nc.vector.tensor_copy(srow[0:1, :], ctx_ps[64:65, :])
                    rb_ps = ps.tile([128, QB], f32, tag="ctx", bufs=2, name="rb_ps")
                    nc.tensor.matmul(
                        rb_ps[:], ones_sb[0:1, 0:128], srow[0:1, :],
                        start=True, stop=True,
                    )
                    nc.vector.reciprocal(out=rb_ps[:], in_=rb_ps[:])
                    nc.vector.tensor_mul(
                        ctxu[pt][hrow, qsl], ctxu[pt][hrow, qsl], rb_ps[hrow, :])

            # ---- O-projection + store for a T-block range ----
            def emit_out_range(tb0, tb1):
                for tb in range(tb0, tb1):
                    tsl = slice(tb * 128, (tb + 1) * 128)
                    for on in range(2):
                        o_ps = ps.tile([128, 512], f32, tag="s2", bufs=3, name="o_ps")
                        for pt in range(2):
                            nc.tensor.matmul(
                                o_ps[:],
                                ctxu[pt][:, tsl],
                                wo_sb[:, pt * D_MODEL + on * 512: pt * D_MODEL + (on + 1) * 512],
                                start=(pt == 0), stop=(pt == 1),
                            )
                        o_sb = big.tile([128, 512], f32, tag="osb", bufs=6, name="o_sb")
                        if on == 0:
                            nc.vector.tensor_copy(o_sb[:], o_ps[:])
                        else:
                            nc.scalar.copy(o_sb[:], o_ps[:])
                        nc.sync.dma_start(out_d[tsl, on * 512: (on + 1) * 512], o_sb[:])

            # lag-1 interleave: exp work reaches ACT early while PE still has
            # projection fill work; O-projection per quarter gives the
            # scheduler PE fill work during later attention blocks
            x0 = emit_x(0)
            nc.sync.dma_start(
                wk_sb[:].rearrange("p (c m) -> p c m", c=8),
                wk_d.rearrange("(c p) m -> p c m", p=128),
            )
            nc.sync.dma_start(
                wv_sb[:].rearrange("p (c m) -> p c m", c=8),
                wv_d.rearrange("(c p) m -> p c m", p=128),
            )
            emit_proj(0, x0)
            nc.sync.dma_start(
                mk_sb[:].rearrange("p (j m) -> p j m", j=4),
                mk_d.rearrange("j p m -> p j m"),
            )
            x1 = emit_x(1)
            emit_proj(1, x1)
            nc.sync.dma_start(
                wo_sb[:].rearrange("p (c m) -> p c m", c=2),
                wo_d.rearrange("(c p) m -> p c m", p=128),
            )
            emit_attn(0)
            emit_proj(2, emit_x(2))
            emit_attn(1)
            emit_proj(3, emit_x(3))
            emit_out_range(0, 4)
            emit_attn(2)
            emit_out_range(4, 8)
            emit_attn(3)
            emit_out_range(8, 16)

    _split_waits(nc, mybir)

    # This walrus build cannot encode EVENT_SEMAPHORE_RANGE_CLEAR ("ISA wrong
    # length"). It only matters for back-to-back kernel reuse of the same
    # loaded NEFF with dirty semaphores; drop it and rely on runtime reset.
    for bb in nc.m.functions[0].blocks:
        bb.instructions = [
            inst for inst in bb.instructions
            if getattr(inst, "op_name", None) != "EVENT_SEMAPHORE_RANGE_CLEAR"
        ]
    return nc


def _get_nc(with_bias=False):
    key = ("nc", with_bias)
    if key not in _CACHE:
        _CACHE[key] = _build_nc(with_bias)
    return _CACHE[key]


def _causal_masks():
    """Additive masks: 0 where k <= q, -1e30 where masked."""
    ql = np.arange(QB)[None, :]
    out = np.empty((4, 128, QB), np.float32)
    for rel in range(4):
        kg = rel * 128 + np.arange(128)[:, None]
        out[rel] = np.where(ql >= kg, 0.0, NEG).astype(np.float32)
    return out


def _augment_wv(wvT_slice, bv_slice):
    """[1024, 256] + [256] -> [1024, 260] + [1, 260] with per-head ones cols."""
    wv_aug = np.zeros((D_MODEL, MPCA), np.float32)
    bv_aug = np.zeros((1, MPCA), np.float32)
    for j in range(HPC):
        wv_aug[:, j * 65: j * 65 + 64] = wvT_slice[:, j * 64: (j + 1) * 64]
        bv_aug[0, j * 65: j * 65 + 64] = bv_slice[j * 64: (j + 1) * 64]
        bv_aug[0, j * 65 + 64] = 1.0
    return wv_aug, bv_aug


def kernel(x, Wq, bq, Wk, bk, Wv, bv, Wo, bo):
    global LAST_RESULTS
    from concourse.bass_utils import run_bass_kernel_spmd

    x = np.asarray(x, np.float32)
    Wq = np.asarray(Wq, np.float32)
    Wk = np.asarray(Wk, np.float32)
    Wv = np.asarray(Wv, np.float32)
    Wo = np.asarray(Wo, np.float32)
    bq = np.asarray(bq, np.float32)
    bk = np.asarray(bk, np.float32)
    bv = np.asarray(bv, np.float32)
    bo = np.asarray(bo, np.float32)

    wqT, wkT, wvT, woT = Wq.T, Wk.T, Wv.T, Wo.T
    masks = _causal_masks()
    xts = [np.ascontiguousarray(x[b].T) for b in range(B)]

    in_maps = []
    for c in range(N_CORES):
        b, hg = c // 4, c % 4
        sl = slice(hg * MPC, (hg + 1) * MPC)
        wv_aug, bv_aug = _augment_wv(wvT[:, sl], bv[sl])
        in_maps.append({
            "xt": xts[b],
            "wq": np.ascontiguousarray(wqT[:, sl]),
            "wk": np.ascontiguousarray(wkT[:, sl]),
            "wv": wv_aug,
            "wo": np.ascontiguousarray(woT[sl, :]),
            "bqr": np.ascontiguousarray(bq[sl].reshape(1, MPC)),
            "bkr": np.ascontiguousarray(bk[sl].reshape(1, MPC)),
            "bv1": bv_aug,
            "ones": np.ones((1, QB), np.float32),
            "masks": masks,
        })

    with_bias = bool(np.any(bq != 0.0) or np.any(bk != 0.0))
    nc = _get_nc(with_bias)
    res = run_bass_kernel_spmd(
        nc, in_maps, list(range(N_CORES)), trace=PROFILE,
    )
    LAST_RESULTS = res

    out = np.zeros((B, T, D_MODEL), np.float32)
    for c in range(N_CORES):
        out[c // 4] += res.results[c]["out"]
    out += bo
    return out

